# revision 31
# baseline (speedup 1.0000x reference)
"""Trainium2 Bass kernel for multi-head causal attention.

Problem: q, k, v of shape [4096, 16, 64] (seq, heads, head_dim) fp32.
  out = softmax(causal(q @ k^T / 8)) @ v, reshaped to [4096, 1024].

Sharding: heads split across 8 NeuronCores (2 heads per core), host concat.

Per-core algorithm ("ribbon" flash-attention, S^T orientation):
  - Stage Q, K via SWDGE cast-DMA (fp32->bf16, seq-major) then hardware
    DMA-transpose into qT/kT [(h,d)=128, seq] (head_dim on partitions).
    V cast-DMAs straight into vplus [128, 32*(64+1)] (ones col fused).
  - All valid S^T columns (causal: block j covers q >= 128j) are packed
    into a contiguous "ribbon" (67584 cols/head) cut into 132 windows of
    512 cols/head. PSUM window tiles [128, 1024] hold h0 in bank0 and h1
    in bank1 so the two heads' K=64 mm1 matmuls (tile_position row
    quadrants) execute CONCURRENTLY on the PE (merged pairs).
  - exp: whole windows are assigned to either the ACT engine (true Exp,
    scale=1/8) or the DVE (Schraudolph bit-trick: int16(round(s*M + B))
    reinterpreted as bf16 ~= exp(s/8), max rel err ~3.3%). The two
    engines run concurrently; G0's windows stay on ACT for accuracy
    (few-key rows lack error cancellation).
  - Diagonal 128x128 chunks get their causal triangle zeroed in-place by
    Pool affine_select on the exp output (both heads in one instr).
  - mm2: P^T chunks [128,128] as stationary weights, vplus[j] streams
    (N=65: 64 V cols + ones col for the softmax denominator), PSUM
    accumulation per (G, head) into po[G%2][h]; ACT/DVE drain po to SBUF
    unnormalized, DMA out [4096, 130] per core.
  - Host divides by the denominator column and concatenates cores.
"""

import numpy as np

SEQ = 4096
NHEAD = 16
HDIM = 64
NCORES = 8
HPC = NHEAD // NCORES  # 2
SCALE = 0.125
LOG2E = 1.4426950408889634
TRICK_M = SCALE * 128.0 * LOG2E  # 23.08312...
TRICK_B = 127.0 * 128.0 - 5.5  # minimax bias, calibrated on host

WIN = 512  # ribbon window width per head
# exp engine pattern after the forced-ACT prefix: A=ACT exp, D=DVE trick
EXP_PATTERN = "AADAD"
ACT_FORCE_WINS = 3  # windows 0..2 (G0 region) always ACT

_NC_CACHE = {}
LAST_RESULT = {}


def _ribbon():
    """Compile-time tables: pieces, per-window mm1 fragments, mm2 chunks.

    Pieces within a G are ordered so piece boundaries align with the 512-col
    windows (kills mm1 fragmentation): a G whose ribbon segment starts at a
    half-window leads with its 256-wide diagonal piece (t=2) to re-align;
    the 384+128 diagonal pair packs one window exactly.
    """
    pieces = []  # (G, j, q0, w, pos)
    pos = 0
    for G in range(8):
        def pc(j):
            t = j - 4 * G
            q0 = max(0, 128 * t)
            return (j, q0, 512 - q0)
        fulls = [pc(j) for j in range(4 * G)]
        t0, t1, t2, t3 = pc(4 * G), pc(4 * G + 1), pc(4 * G + 2), pc(4 * G + 3)
        if pos % WIN == 0:
            order = fulls + [t0, t1, t3, t2]
        else:
            order = [t2] + fulls + [t0, t1, t3]
        for (j, q0, w) in order:
            pieces.append((G, j, q0, w, pos))
            pos += w
    nwin = pos // WIN
    assert pos % WIN == 0
    frags = [[] for _ in range(nwin)]  # (off, G, j, q0f, fw)
    chunks = [[] for _ in range(nwin)]  # (off, G, j, c, diag, first, last)
    gfirst = {}
    glast = {}
    for (G, j, q0, w, p) in pieces:
        left = 0
        while left < w:
            win = (p + left) // WIN
            off = (p + left) % WIN
            fw = min(w - left, WIN - off)
            frags[win].append((off, G, j, q0 + left, fw))
            left += fw
        for ci in range(w // 128):
            rp = p + 128 * ci
            qg = q0 + 128 * ci
            key = (G, j, qg // 128)
            if G not in gfirst:
                gfirst[G] = key
            glast[G] = key
            chunks[rp // WIN].append(
                (rp % WIN, G, j, qg // 128, (G * 512 + qg) == j * 128)
            )
    return nwin, frags, chunks, gfirst, glast


def build_attention_nc():
    import concourse.bass as bass
    import concourse.mybir as mybir
    import concourse.tile as tile

    f32 = mybir.dt.float32
    bf16 = mybir.dt.bfloat16
    i16 = mybir.dt.int16
    Exp = mybir.ActivationFunctionType.Exp
    Copy = mybir.ActivationFunctionType.Copy

    nwin, frags, chunks, gfirst, glast = _ribbon()

    # mm2 deferral: 2 windows normally; 4 for windows containing a G's
    # first chunks, so the single-buffered po drain of the previous G gets
    # a head start before the next G's start=True bank wipe executes.
    # (PSUM accumulation between start and stop is order-insensitive.)
    defer = [2] * nwin
    gwins = {}
    for wi in range(nwin):
        for (off, G, j, c, diag) in chunks[wi]:
            gwins.setdefault(G, []).append(wi)
    # uniform deferral measured fastest; the first-window bump variant
    # (defer 4 for each G's first two windows) paced worse on HW
    gcount = {
        G: sum(1 for wi in set(ws) for ch in chunks[wi] if ch[1] == G)
        for G, ws in gwins.items()
    }
    due = [[] for _ in range(nwin + 5)]
    for wi in range(nwin):
        due[wi + defer[wi]].append(wi)

    def assign(w):
        if w < ACT_FORCE_WINS:
            return "A"
        return EXP_PATTERN[(w - ACT_FORCE_WINS) % len(EXP_PATTERN)]

    nc = bass.Bass()
    q = nc.dram_tensor("q", [SEQ, HPC, HDIM], f32, kind="ExternalInput").ap()
    k = nc.dram_tensor("k", [SEQ, HPC, HDIM], f32, kind="ExternalInput").ap()
    v = nc.dram_tensor("v", [SEQ, HPC, HDIM], f32, kind="ExternalInput").ap()
    # per-core raw output: per 128-row block, per chunk c: h0 64+den | h1 64+den
    o = nc.dram_tensor("o", [SEQ, HPC * (HDIM + 1)], f32, kind="ExternalOutput").ap()

    with tile.TileContext(nc) as tc:
        with (
            tc.tile_pool(name="persist", bufs=1) as persist,
            tc.tile_pool(name="stage", bufs=4) as stage_pool,
            tc.tile_pool(name="pexp", bufs=6) as pexp_pool,
            tc.tile_pool(name="outp", bufs=2) as out_pool,
            tc.tile_pool(name="pwin", bufs=3, space="PSUM") as pwin_pool,
            tc.tile_pool(name="ppo", bufs=1, space="PSUM") as po_pool,
        ):
            # transposed Q/K in fine-grained 512-col unit tiles (8 each) so
            # consumers only dep on the unit they read (no false stalls)
            kT = [persist.tile([128, 512], bf16, tag=f"kT{i}", name=f"kT{i}") for i in range(8)]
            qT = [persist.tile([128, 512], bf16, tag=f"qT{i}", name=f"qT{i}") for i in range(8)]
            # vplus unit: 8 k-blocks x (64 V cols + ones col) per head
            vplus = [
                [
                    persist.tile([128, 8 * 65], bf16, tag=f"v{h}_{i}", name=f"v{h}_{i}")
                    for i in range(4)
                ]
                for h in range(HPC)
            ]
            # single-buffered output accumulators (one bank per head); the
            # next G's start=True bank wipe is held off by its drain dep
            po = [po_pool.tile([128, 260], f32, tag=f"po{h}", name=f"po{h}") for h in range(HPC)]

            # ---- staging machinery ------------------------------------------
            # cast-DMA (SWDGE, Pool queue) into a seq-major stage tile, then
            # PE transposes (into a borrowed wt-ring PSUM slot, bf16-bitcast)
            # and one batched DVE copy PSUM->SBUF into kT/qT. Triggers and
            # transposes are emitted at separate loop points so the in-order
            # PE queue never waits on an in-flight stage DMA.
            identity = persist.tile([128, 128], f32, tag="identity")
            from concourse.masks import make_identity

            stage_tiles = {}

            def stage_trigger(kind, ci):
                """Queue the staging DMA for an 8-block chunk ci (0..3).

                V: SWDGE cast-DMA on the Pool queue (only user of SWDGE).
                K/Q: plain fp32 DMA on the SP queue (HWDGE) — the fp32->bf16
                cast happens in the DVE PSUM->SBUF copy after PE transpose.
                """
                if kind == "v":
                    for h in range(HPC):
                        vt = vplus[h][ci]
                        vv = vt.rearrange("p (t f) -> p t f", f=HDIM + 1)
                        nc.gpsimd.memset(vv[:, :, HDIM : HDIM + 1], 1.0)
                        nc.gpsimd.dma_start(
                            out=vv[:, :, 0:HDIM],
                            in_=v[:, h, :].rearrange("(t p) d -> p t d", p=128)[
                                :, ci * 8 : (ci + 1) * 8, :
                            ],
                        )
                    return
                src = k if kind == "k" else q
                st = stage_pool.tile([128, 1024], f32, tag="st", name="st")
                stage_tiles[(kind, ci)] = st
                nc.sync.dma_start(
                    out=st.rearrange("p (t x) -> p t x", x=128),
                    in_=src.rearrange("(t p) h d -> p t (h d)", p=128)[
                        :, ci * 8 : (ci + 1) * 8, :
                    ],
                )

            def stage_trigger_half(kind, ui):
                """Fine 4-block trigger for chunk-0 units (startup)."""
                src = k if kind == "k" else q
                st = stage_pool.tile([128, 512], f32, tag="st0", name="st0")
                stage_tiles[(kind, ui, "half")] = st
                nc.sync.dma_start(
                    out=st.rearrange("p (t x) -> p t x", x=128),
                    in_=src.rearrange("(t p) h d -> p t (h d)", p=128)[
                        :, ui * 4 : (ui + 1) * 4, :
                    ],
                )

            def stage_transpose(kind, ui, copy_eng="dve"):
                """Transpose+cast one 4-block unit ui (0..7)."""
                dstT = kT if kind == "k" else qT
                if (kind, ui, "half") in stage_tiles:
                    st = stage_tiles[(kind, ui, "half")]
                    half = 0
                else:
                    st = stage_tiles[(kind, ui // 2)]
                    half = (ui % 2) * 512
                tr = pwin_pool.tile([128, 1024], f32, tag="wt", name="tr")
                for t4 in range(4):
                    nc.tensor.transpose(
                        tr[:, t4 * 128 : (t4 + 1) * 128],
                        st[:, half + t4 * 128 : half + (t4 + 1) * 128],
                        identity[:],
                    )
                if copy_eng == "act":
                    nc.scalar.activation(out=dstT[ui][:], in_=tr[:, 0:512], func=Copy)
                else:
                    nc.vector.tensor_copy(dstT[ui][:], tr[:, 0:512])

            # need-ordered staging schedule: emission window -> events.
            # With the aligned piece order, k/q/v unit u is first needed at
            # the start of G=u: windows [0,2,9,19,34,52,75,101]. All K/Q
            # DMA triggers are issued up front (SP queue, ~12us of DMA);
            # only the PE transposes + DVE cast-copies are spread out.
            # transpose events sit on AA-pattern windows (both this and the
            # next window on ACT) so the DVE cast-copies use DVE idle time
            STAGE_EVENTS = {
                5: [("k", 2, "x"), ("q", 2, "x")],
                15: [("k", 3, "x"), ("q", 3, "x")],
                24: [("v", 2, "t")],
                27: [("k", 4, "x"), ("q", 4, "x")],
                45: [("k", 5, "x"), ("q", 5, "x")],
                64: [("v", 3, "t")],
                70: [("k", 6, "x"), ("q", 6, "x")],
                95: [("k", 7, "x"), ("q", 7, "x")],
            }

            # identity first (no DMA dependency — unblocks the transposes),
            # then chunk-0 4-block triggers (startup critical path), then
            # the remaining chunk triggers fill the SP queue
            make_identity(nc, identity[:])
            stage_trigger_half("k", 0)
            stage_trigger_half("q", 0)
            stage_trigger("v", 0)
            stage_trigger_half("k", 1)
            stage_trigger_half("q", 1)
            stage_trigger("v", 1)
            for ci in (1, 2, 3):
                stage_trigger("k", ci)
                stage_trigger("q", ci)
            stage_transpose("k", 0)
            stage_transpose("q", 0)
            stage_transpose("k", 1)
            stage_transpose("q", 1)

            # ---- main ribbon loop ----
            pexp_tiles = [None] * nwin
            for w in range(nwin + 5):
                for ev in STAGE_EVENTS.get(w, ()):
                    kind, ci, phase = ev
                    if phase == "t":
                        stage_trigger(kind, ci)
                    else:
                        ce = "act" if (w < nwin and assign(w) == "D") else "dve"
                        stage_transpose(kind, ci, copy_eng=ce)
                if w < nwin:
                    # mm1: merged head pairs into the window PSUM tile
                    wt = pwin_pool.tile([128, 1024], f32, tag="wt", name="wt")
                    for (off, G, j, q0f, fw) in frags[w]:
                        qlo = G * 512 + q0f
                        for h in range(HPC):
                            nc.tensor.matmul(
                                wt[:, 512 * h + off : 512 * h + off + fw],
                                lhsT=kT[j // 4][
                                    64 * h : 64 * h + 64, (j % 4) * 128 : (j % 4 + 1) * 128
                                ],
                                rhs=qT[qlo // 512][
                                    64 * h : 64 * h + 64, qlo % 512 : qlo % 512 + fw
                                ],
                                start=True,
                                stop=True,
                                tile_position=(h * 64, 0),
                                skip_group_check=True,
                            )
                    # exp: whole window on one engine
                    pe_t = pexp_pool.tile([128, 1024], bf16, tag="pexp", name="pexp")
                    pexp_tiles[w] = pe_t
                    if assign(w) == "A":
                        nc.scalar.activation(out=pe_t[:], in_=wt[:], func=Exp, scale=SCALE)
                    else:
                        nc.vector.tensor_scalar(
                            out=pe_t[:].bitcast(i16),
                            in0=wt[:],
                            scalar1=float(TRICK_M),
                            scalar2=float(TRICK_B),
                            op0=mybir.AluOpType.mult,
                            op1=mybir.AluOpType.add,
                        )
                    # causal triangle on diagonal chunks (both heads, one instr)
                    for (off, G, j, c, diag) in chunks[w]:
                        if not diag:
                            continue
                        nc.gpsimd.affine_select(
                            out=pe_t[:].rearrange("p (h x) -> p h x", h=2)[:, :, off : off + 128],
                            in_=pe_t[:].rearrange("p (h x) -> p h x", h=2)[:, :, off : off + 128],
                            compare_op=mybir.AluOpType.is_ge,
                            fill=0.0,
                            base=0,
                            pattern=[[0, 2], [1, 128]],
                            channel_multiplier=-1,
                        )
                # mm2 deferred per the `due` schedule (2 or 4 windows)
                for wm in due[w]:
                    pv = pexp_tiles[wm]
                    # PSUM accumulation: exactly ONE group per (G,h) bank —
                    # a second start=True while the group is open destroys
                    # the open partial sums (verified on HW). po is single-
                    # buffered, so drain immediately when a G completes
                    # (before the next G's start=True wipes the bank).
                    for (off, G, j, c, diag) in chunks[wm]:
                        for h in range(HPC):
                            nc.tensor.matmul(
                                po[h][:, c * 65 : c * 65 + 65],
                                lhsT=pv[:, 512 * h + off : 512 * h + off + 128],
                                rhs=vplus[h][j // 8][:, (j % 8) * 65 : (j % 8) * 65 + 65],
                                start=(gfirst[G] == (G, j, c)),
                                stop=(glast[G] == (G, j, c)),
                                skip_group_check=True,
                            )
                        gcount[G] -= 1
                        if gcount[G] == 0:
                            # all of G's chunks emitted: drain NOW, before
                            # any later G's start=True wipes the po banks.
                            # Drain engine = opposite of this window's exp.
                            ob = out_pool.tile([128, 4 * 130], f32, tag="ob", name="ob")
                            obv = ob.rearrange("p (c hf) -> p c hf", hf=130)
                            drains = (
                                (0, obv[:, :, 0:65]),
                                (1, obv[:, :, 65:130]),
                            )
                            if w < nwin and assign(w) == "A":
                                for hh, dst in drains:
                                    nc.vector.tensor_copy(
                                        dst,
                                        po[hh][:, 0:260].rearrange("p (c f) -> p c f", f=65),
                                    )
                            else:
                                for hh, dst in drains:
                                    nc.scalar.activation(
                                        out=dst,
                                        in_=po[hh][:, 0:260].rearrange("p (c f) -> p c f", f=65),
                                        func=Copy,
                                    )
                            nc.sync.dma_start(
                                out=o[G * 512 : (G + 1) * 512, :].rearrange(
                                    "(c p) f -> p c f", p=128
                                ),
                                in_=obv,
                            )
    _split_multi_waits(nc)
    return nc


def _split_multi_waits(nc):
    """Walrus accepts at most one sync-wait per instruction on this
    toolchain; hoist extras into single-wait NoOps on the same queue."""
    import concourse.mybir as mybir

    nsplit = 0
    for blk in nc.m.functions[0].blocks:
        newl = []
        for ins in blk.instructions:
            si = getattr(ins, "sync_info", None)
            if si is not None and si.on_wait and len(si.on_wait) > 1:
                waits = list(si.on_wait)
                for wt in waits[:-1]:
                    newl.append(
                        mybir.InstNoOp(
                            name=f"{ins.name}-wsplit{nsplit}",
                            sync_info=mybir.SyncInfo(on_wait=[wt], on_update=[]),
                            bass_nofuse=True,
                            engine=ins.engine,
                            ins=[],
                            outs=[],
                        )
                    )
                    nsplit += 1
                ins.sync_info = mybir.SyncInfo(
                    on_wait=[waits[-1]], on_update=list(si.on_update or [])
                )
            newl.append(ins)
        blk.instructions = newl
    return nsplit


def _ensure_ntff_hook():
    """Provide antenv.axon_hooks if the image lacks it (trace path)."""
    import sys
    import types

    try:
        import antenv.axon_hooks  # noqa: F401

        return
    except ImportError:
        pass
    mod = types.ModuleType("antenv.axon_hooks")
    state = {"hook": None}
    mod.set_axon_ntff_profile_hook = lambda h: state.__setitem__("hook", h)
    mod.get_axon_ntff_profile_hook = lambda: state["hook"]
    try:
        from trn_agent_boot.trn_boot import _ntff_profile_via_ctypes

        state["hook"] = _ntff_profile_via_ctypes("/opt/axon/libaxon_pjrt.so")
    except Exception:
        state["hook"] = None
    sys.modules["antenv.axon_hooks"] = mod


def kernel(q, k, v):
    """Full-input entry point: q, k, v [4096, 16, 64] fp32 -> [4096, 1024]."""
    import sys

    if "/opt/trn_rl_repo" not in sys.path:
        sys.path.insert(0, "/opt/trn_rl_repo")
    _ensure_ntff_hook()
    from concourse.bass_utils import run_bass_kernel_spmd

    q = np.asarray(q, dtype=np.float32)
    k = np.asarray(k, dtype=np.float32)
    v = np.asarray(v, dtype=np.float32)

    if "nc" not in _NC_CACHE:
        _NC_CACHE["nc"] = build_attention_nc()
    nc = _NC_CACHE["nc"]

    in_maps = []
    for c in range(NCORES):
        hs = slice(c * HPC, (c + 1) * HPC)
        in_maps.append(
            {
                "q": np.ascontiguousarray(q[:, hs, :]),
                "k": np.ascontiguousarray(k[:, hs, :]),
                "v": np.ascontiguousarray(v[:, hs, :]),
            }
        )
    res = run_bass_kernel_spmd(nc, in_maps, core_ids=list(range(NCORES)))
    LAST_RESULT["exec_time_ns"] = res.exec_time_ns
    try:
        iat = res.instructions_and_trace
        LAST_RESULT["trace_path"] = iat[1] if iat else None
    except Exception:
        LAST_RESULT["trace_path"] = None
    outs = []
    for c in range(NCORES):
        raw = res.results[c]["o"]  # [4096, 130]
        for h in range(HPC):
            num = raw[:, h * 65 : h * 65 + 64]
            den = raw[:, h * 65 + 64 : h * 65 + 65]
            outs.append(num / den)
    return np.concatenate(outs, axis=1)
